# revision 10
# baseline (speedup 1.0000x reference)
"""RoIPool (quantized max pool, torchvision semantics) Bass kernel for TRN2.

Contract: kernel(features, rois) -> (B, N, C, 7, 7) float32.
  features: (2, 128, 56, 56) f32, rois: (2, 128, 4) f32 cxcywh normalized.

Sharding: 8 cores = 2 images x 4 roi-chunks of 32. Each core gets its
image's full feature map and 32 rois, and computes (C, 32*49) outputs.

Per-core algorithm (channels in partitions):
  1. Build 8 2D sparse max tables T[kw in 0..3][khidx in 0..1]:
     T[kw][khidx][h, w] = max over f[h : h+hwin, w : w+2^kw], hwin in {1,4}.
     8 elementwise-max passes (h-pyramid once, then two w-chains).
  2. Compute all bin boundaries on-device from rois, bit-exact with the
     jax reference (magic-number floor/ceil, /7 via lookup table).
  3. Each output pixel = max of 6 gathered table entries:
     2 w-reads (RMQ, kw = floor(log2(zw)) <= 3) x 3 h-reads (windows of
     1 for zh<4, windows of 4 for zh in 4..10, covering [hs, he)).
     Empty bins gather a zeroed slot -> output 0.
"""

import os
import sys

import numpy as np

sys.path.insert(0, "/opt/trn_rl_repo")

import concourse.bacc as bacc
import concourse.bass as bass
import concourse.mybir as mybir
import concourse.tile as tile

f32 = mybir.dt.float32
i16 = mybir.dt.int16
ALU = mybir.AluOpType
AX = mybir.AxisListType

P = 128
H = W = 56
HW = H * W              # 3136
NT = 8                  # tables: t = kw*2 + khidx (khidx: h-window 1 or 4)
ZSLOT = NT * HW         # 25088: zero slot for empty bins
TABF = NT * HW + 4      # table buffer free size (pad to num_idxs%4 rules)
NROI = 32               # rois per core
NOUT = 2 * 7 * 7 * 16   # 1568 outputs per partition: i=(nhi, ph, pw, nlo)
MAGIC = 12582912.0      # 1.5 * 2^23: (x+MAGIC)-MAGIC = round-to-nearest-int


def _consts_array():
    """(P, 160) f32: [0:7) pw, [8:15) pw+1, [16:80) iota64, [80:144) maxL/7."""
    c = np.zeros((P, 160), np.float32)
    c[:, 0:7] = np.arange(7, dtype=np.float32)
    c[:, 8:15] = np.arange(1, 8, dtype=np.float32)
    c[:, 16:80] = np.arange(64, dtype=np.float32)
    lv = np.maximum(np.arange(64, dtype=np.float32), np.float32(1.0))
    c[:, 80:144] = lv / np.float32(7.0)
    return c


def build_nc():
    nc = bacc.Bacc(None, target_bir_lowering=False, debug=False)
    feat_d = nc.declare_dram_parameter("features", [P, H, W], f32, isOutput=False)
    rois_d = nc.declare_dram_parameter("rois", [NROI, 4], f32, isOutput=False)
    out_d = nc.declare_dram_parameter("out", [P, NOUT], f32, isOutput=True)
    const_d = nc.inline_tensor(_consts_array(), name="kconsts")

    with tile.TileContext(nc) as tc:
        with tc.tile_pool(name="pool", bufs=1) as pool:
            _emit(nc, tc, pool, feat_d, rois_d, out_d, const_d)
    nc.finalize()
    return nc


def _emit(nc, tc, pool, feat_d, rois_d, out_d, const_d):
    any_ = nc.any
    vec = nc.vector

    tab = pool.tile([P, TABF], f32, tag="tab")
    cst = pool.tile([P, 160], f32, tag="cst")
    R = pool.tile([P, 8], f32, tag="R")      # (nhi, [cx cy w h]) * 56
    V = pool.tile([P, 8], f32, tag="V")      # (nhi, [x1 y1 x2 y2]) raw
    VT = pool.tile([P, 8], f32, tag="VT")    # V + 0.5
    RN = pool.tile([P, 8], f32, tag="RN")
    FX = pool.tile([P, 8], f32, tag="FX")
    VR = pool.tile([P, 8], f32, tag="VR")    # rounded coords
    LT = pool.tile([P, 4], f32, tag="LT")    # (nhi, ax) box size L (clamped)
    BW = pool.tile([P, 4], f32, tag="BW")    # (nhi, ax) bin size L/7
    MQ = pool.tile([P, 256], f32, tag="MQ")  # lookup scratch
    S0 = pool.tile([P, 28], f32, tag="S0")   # (nhi, ax, 7) raw starts
    SN = pool.tile([P, 28], f32, tag="SN")
    SF = pool.tile([P, 28], f32, tag="SF")
    WS = pool.tile([P, 28], f32, tag="WS")   # clipped starts
    E0 = pool.tile([P, 28], f32, tag="E0")
    EN = pool.tile([P, 28], f32, tag="EN")
    EF = pool.tile([P, 28], f32, tag="EF")
    WE = pool.tile([P, 28], f32, tag="WE")   # clipped ends
    Z = pool.tile([P, 28], f32, tag="Z")     # sizes
    M2 = pool.tile([P, 28], f32, tag="M2")
    M4 = pool.tile([P, 28], f32, tag="M4")
    M3 = pool.tile([P, 28], f32, tag="M3")
    M8 = pool.tile([P, 28], f32, tag="M8")
    EM = pool.tile([P, 28], f32, tag="EM")   # empty flags
    K2 = pool.tile([P, 28], f32, tag="K2")   # m2+m4 (kh on h-slice)
    KW = pool.tile([P, 28], f32, tag="KW")   # m2+m4+m8 (kw on w-slice)
    P2 = pool.tile([P, 28], f32, tag="P2")   # 2^kw
    WB = pool.tile([P, 14], f32, tag="WB")   # (nhi, 7) second w-read
    H1S = pool.tile([P, 14], f32, tag="H1S")
    H2S = pool.tile([P, 14], f32, tag="H2S")
    SCR = pool.tile([P, HW], f32, tag="SCR")
    SM = pool.tile([P, 14], f32, tag="SM")
    MD = pool.tile([P, 14], f32, tag="MD")
    MN = pool.tile([P, 14], f32, tag="MN")
    MF = pool.tile([P, 14], f32, tag="MF")
    MD2 = pool.tile([P, 14], f32, tag="MD2")
    DD = pool.tile([P, 14], f32, tag="DD")
    H1 = pool.tile([P, 14], f32, tag="H1")
    T0 = pool.tile([P, 14], f32, tag="T0")
    D2 = pool.tile([P, 14], f32, tag="D2")
    H2 = pool.tile([P, 14], f32, tag="H2")
    AA = pool.tile([P, 14], f32, tag="AA")   # w-read idx contribution a
    AB = pool.tile([P, 14], f32, tag="AB")   # w-read idx contribution b
    BH = pool.tile([P, 42], f32, tag="BH")   # (j, nhi, 7) h-read contribs
    BT = pool.tile([P, 42], f32, tag="BT")
    IDXF = pool.tile([P, 6 * 98], f32, tag="IDXF")
    EC = pool.tile([P, 98], f32, tag="EC")
    OM = pool.tile([P, 98], f32, tag="OM")
    ZC = pool.tile([P, 98], f32, tag="ZC")
    IDXU = pool.tile([P, 6 * 98], i16, tag="IDXU")
    pl = [pool.tile([P, NOUT], f32, name=f"pl{i}", tag=f"pl{i}") for i in range(3)]
    acc = pool.tile([P, NOUT], f32, tag="acc")

    # ---- DMAs in
    nc.sync.dma_start(out=cst[:, :], in_=const_d[:, :])
    nc.sync.dma_start(
        out=tab[:, 0:HW].rearrange("p (h w) -> p h w", h=H),
        in_=feat_d[:, :, :],
    )
    # rois (32,4) -> R: partition p holds rois p%16 (nhi=0) and 16+p%16
    rr = rois_d[:, :].rearrange("(nhi nlo) f -> nlo nhi f", nlo=16)
    for g in range(8):
        nc.sync.dma_start(
            out=R[16 * g : 16 * g + 16, :].rearrange("p (a b) -> p a b", a=2),
            in_=rr,
        )
    vec.memset(tab[:, ZSLOT:TABF], 0.0)

    def v3(t, a, b):
        return t[:, 0 : a * b].rearrange("p (a b) -> p a b", a=a)

    Rv = v3(R, 2, 4)
    Vv = v3(V, 2, 4)

    # ---- rois * 56 ; corners (bitwise identical to reference ops)
    vec.tensor_scalar(out=R[:, :], in0=R[:, :], scalar1=56.0, scalar2=None,
                      op0=ALU.mult)
    stt = vec.scalar_tensor_tensor
    stt(out=Vv[:, :, 0:1], in0=Rv[:, :, 2:3], scalar=-0.5,
        in1=Rv[:, :, 0:1], op0=ALU.mult, op1=ALU.add)
    stt(out=Vv[:, :, 1:2], in0=Rv[:, :, 3:4], scalar=-0.5,
        in1=Rv[:, :, 1:2], op0=ALU.mult, op1=ALU.add)
    stt(out=Vv[:, :, 2:3], in0=Rv[:, :, 2:3], scalar=0.5,
        in1=Rv[:, :, 0:1], op0=ALU.mult, op1=ALU.add)
    stt(out=Vv[:, :, 3:4], in0=Rv[:, :, 3:4], scalar=0.5,
        in1=Rv[:, :, 1:2], op0=ALU.mult, op1=ALU.add)
    # rnd(v) = floor(v + 0.5): magic round-to-nearest, then fix down
    vec.tensor_scalar(out=VT[:, :], in0=V[:, :], scalar1=0.5, scalar2=None,
                      op0=ALU.add)
    vec.tensor_scalar(out=RN[:, :], in0=VT[:, :], scalar1=MAGIC, scalar2=MAGIC,
                      op0=ALU.add, op1=ALU.subtract)
    vec.tensor_tensor(out=FX[:, :], in0=RN[:, :], in1=VT[:, :], op=ALU.is_gt)
    vec.tensor_tensor(out=VR[:, :], in0=RN[:, :], in1=FX[:, :], op=ALU.subtract)

    VRv = v3(VR, 2, 4)
    LTv = v3(LT, 2, 2)
    # L = (x2 + 1) - x1 per axis, clamp to 63 for lookup
    stt(out=LTv[:, :, 0:1], in0=VRv[:, :, 2:3], scalar=1.0,
        in1=VRv[:, :, 0:1], op0=ALU.add, op1=ALU.subtract)
    stt(out=LTv[:, :, 1:2], in0=VRv[:, :, 3:4], scalar=1.0,
        in1=VRv[:, :, 1:2], op0=ALU.add, op1=ALU.subtract)
    vec.tensor_scalar(out=LT[:, :], in0=LT[:, :], scalar1=63.0, scalar2=None,
                      op0=ALU.min)
    # bw = (max(L,1)/7)[L] via one-hot match + dot
    iota_b = cst[:, 16:80].unsqueeze(1).unsqueeze(1).broadcast_to((P, 2, 2, 64))
    lt_b = v3(LT, 2, 2).unsqueeze(3).broadcast_to((P, 2, 2, 64))
    mq_v = MQ[:, :].rearrange("p (a b c) -> p a b c", a=2, b=2)
    vec.tensor_tensor(out=mq_v, in0=iota_b, in1=lt_b, op=ALU.is_equal)
    bwt_b = cst[:, 80:144].unsqueeze(1).unsqueeze(1).broadcast_to((P, 2, 2, 64))
    vec.tensor_tensor(out=mq_v, in0=mq_v, in1=bwt_b, op=ALU.mult)
    vec.tensor_reduce(out=v3(BW, 2, 2), in_=mq_v, axis=AX.X, op=ALU.add)

    # ---- starts / ends, (nhi, ax, 7) tiles
    bw_b = v3(BW, 2, 2).unsqueeze(3).broadcast_to((P, 2, 2, 7))
    pw_b = cst[:, 0:7].unsqueeze(1).unsqueeze(1).broadcast_to((P, 2, 2, 7))
    pw1_b = cst[:, 8:15].unsqueeze(1).unsqueeze(1).broadcast_to((P, 2, 2, 7))
    xy_b = v3(VR, 2, 4)[:, :, 0:2].unsqueeze(3).broadcast_to((P, 2, 2, 7))

    def v4(t):
        return t[:, :].rearrange("p (a b c) -> p a b c", a=2, b=2)

    vec.tensor_tensor(out=v4(S0), in0=pw_b, in1=bw_b, op=ALU.mult)
    vec.tensor_scalar(out=SN[:, :], in0=S0[:, :], scalar1=MAGIC, scalar2=MAGIC,
                      op0=ALU.add, op1=ALU.subtract)
    vec.tensor_tensor(out=SF[:, :], in0=SN[:, :], in1=S0[:, :], op=ALU.is_gt)
    vec.tensor_tensor(out=SF[:, :], in0=SN[:, :], in1=SF[:, :], op=ALU.subtract)
    vec.tensor_tensor(out=v4(WS), in0=v4(SF), in1=xy_b, op=ALU.add)
    vec.tensor_scalar(out=WS[:, :], in0=WS[:, :], scalar1=0.0, scalar2=56.0,
                      op0=ALU.max, op1=ALU.min)

    vec.tensor_tensor(out=v4(E0), in0=pw1_b, in1=bw_b, op=ALU.mult)
    vec.tensor_scalar(out=EN[:, :], in0=E0[:, :], scalar1=MAGIC, scalar2=MAGIC,
                      op0=ALU.add, op1=ALU.subtract)
    vec.tensor_tensor(out=EF[:, :], in0=EN[:, :], in1=E0[:, :], op=ALU.is_lt)
    vec.tensor_tensor(out=EF[:, :], in0=EN[:, :], in1=EF[:, :], op=ALU.add)
    vec.tensor_tensor(out=v4(WE), in0=v4(EF), in1=xy_b, op=ALU.add)
    vec.tensor_scalar(out=WE[:, :], in0=WE[:, :], scalar1=0.0, scalar2=56.0,
                      op0=ALU.max, op1=ALU.min)

    # ---- sizes, levels, reads
    vec.tensor_tensor(out=Z[:, :], in0=WE[:, :], in1=WS[:, :], op=ALU.subtract)
    vec.tensor_scalar(out=M2[:, :], in0=Z[:, :], scalar1=2.0, scalar2=None,
                      op0=ALU.is_ge)
    vec.tensor_scalar(out=M3[:, :], in0=Z[:, :], scalar1=3.0, scalar2=None,
                      op0=ALU.is_ge)
    vec.tensor_scalar(out=M4[:, :], in0=Z[:, :], scalar1=4.0, scalar2=None,
                      op0=ALU.is_ge)
    vec.tensor_scalar(out=M8[:, :], in0=Z[:, :], scalar1=8.0, scalar2=None,
                      op0=ALU.is_ge)
    vec.tensor_scalar(out=EM[:, :], in0=Z[:, :], scalar1=0.0, scalar2=None,
                      op0=ALU.is_le)
    vec.tensor_tensor(out=K2[:, :], in0=M2[:, :], in1=M4[:, :], op=ALU.add)
    vec.tensor_tensor(out=KW[:, :], in0=K2[:, :], in1=M8[:, :], op=ALU.add)
    stt(out=P2[:, :], in0=M4[:, :], scalar=2.0, in1=M2[:, :],
        op0=ALU.mult, op1=ALU.add)
    stt(out=P2[:, :], in0=M8[:, :], scalar=4.0, in1=P2[:, :],
        op0=ALU.mult, op1=ALU.add)
    vec.tensor_scalar(out=P2[:, :], in0=P2[:, :], scalar1=1.0, scalar2=None,
                      op0=ALU.add)

    def wsl(t):
        return v4(t)[:, :, 0, :]

    def hsl(t):
        return v4(t)[:, :, 1, :]

    def h2d(t):
        return t[:, :].rearrange("p (a c) -> p a c", a=2)

    # second w-read: wb = we_w - 2^kw
    vec.tensor_tensor(out=h2d(WB), in0=wsl(WE), in1=wsl(P2), op=ALU.subtract)
    # h-reads: h0 = hs ; for zh<4 (win1): h1 = hs+m2, h2 = hs+m2+m3
    #          for zh>=4 (win4): h1 = floor((hs+he)/2)-2, h2 = he-4
    vec.tensor_tensor(out=h2d(H1S), in0=hsl(WS), in1=hsl(M2), op=ALU.add)
    vec.tensor_tensor(out=h2d(H2S), in0=h2d(H1S), in1=hsl(M3), op=ALU.add)
    vec.tensor_tensor(out=h2d(SM), in0=hsl(WS), in1=hsl(WE), op=ALU.add)
    vec.tensor_scalar(out=MD[:, :], in0=SM[:, :], scalar1=0.5, scalar2=None,
                      op0=ALU.mult)
    vec.tensor_scalar(out=MN[:, :], in0=MD[:, :], scalar1=MAGIC, scalar2=MAGIC,
                      op0=ALU.add, op1=ALU.subtract)
    vec.tensor_tensor(out=MF[:, :], in0=MN[:, :], in1=MD[:, :], op=ALU.is_gt)
    vec.tensor_tensor(out=MF[:, :], in0=MN[:, :], in1=MF[:, :], op=ALU.subtract)
    vec.tensor_scalar(out=MD2[:, :], in0=MF[:, :], scalar1=2.0, scalar2=None,
                      op0=ALU.subtract)
    vec.tensor_tensor(out=DD[:, :], in0=MD2[:, :], in1=H1S[:, :],
                      op=ALU.subtract)
    vec.tensor_tensor(out=h2d(DD), in0=h2d(DD), in1=hsl(M4), op=ALU.mult)
    vec.tensor_tensor(out=H1[:, :], in0=H1S[:, :], in1=DD[:, :], op=ALU.add)
    vec.tensor_scalar(out=h2d(T0), in0=hsl(WE), scalar1=4.0, scalar2=None,
                      op0=ALU.subtract)
    vec.tensor_tensor(out=h2d(D2), in0=h2d(T0), in1=h2d(H2S), op=ALU.subtract)
    vec.tensor_tensor(out=h2d(D2), in0=h2d(D2), in1=hsl(M4), op=ALU.mult)
    vec.tensor_tensor(out=h2d(H2), in0=h2d(H2S), in1=h2d(D2), op=ALU.add)

    # ---- index contributions: idx = (kw*3 + kh)*3136 + h*56 + w
    stt(out=h2d(AA), in0=wsl(KW), scalar=float(2 * HW), in1=wsl(WS),
        op0=ALU.mult, op1=ALU.add)
    stt(out=h2d(AB), in0=wsl(KW), scalar=float(2 * HW), in1=h2d(WB),
        op0=ALU.mult, op1=ALU.add)
    hj = [hsl(WS), h2d(H1), h2d(H2)]
    for j in range(3):
        bt_j = BT[:, 14 * j : 14 * j + 14].rearrange("p (a c) -> p a c", a=2)
        bh_j = BH[:, 14 * j : 14 * j + 14].rearrange("p (a c) -> p a c", a=2)
        vec.tensor_scalar(out=bt_j, in0=hj[j], scalar1=56.0, scalar2=None,
                          op0=ALU.mult)
        stt(out=bh_j, in0=hsl(M4), scalar=float(HW), in1=bt_j,
            op0=ALU.mult, op1=ALU.add)

    # ---- assemble idx planes (r = i*3 + j), col s = nhi*49 + ph*7 + pw
    a_t = [AA, AB]
    for i in range(2):
        a_b = h2d(a_t[i]).unsqueeze(2).broadcast_to((P, 2, 7, 7))
        for j in range(3):
            b_b = BH[:, 14 * j : 14 * j + 14] \
                .rearrange("p (a c) -> p a c", a=2) \
                .unsqueeze(3).broadcast_to((P, 2, 7, 7))
            r = i * 3 + j
            out_r = IDXF[:, 98 * r : 98 * r + 98].rearrange(
                "p (a h w) -> p a h w", a=2, h=7)
            vec.tensor_tensor(out=out_r, in0=a_b, in1=b_b, op=ALU.add)
    # empty overlay -> zero slot
    ew_b = wsl(EM).unsqueeze(2).broadcast_to((P, 2, 7, 7))
    eh_b = hsl(EM).unsqueeze(3).broadcast_to((P, 2, 7, 7))
    ec_v = EC[:, :].rearrange("p (a h w) -> p a h w", a=2, h=7)
    vec.tensor_tensor(out=ec_v, in0=ew_b, in1=eh_b, op=ALU.max)
    vec.tensor_scalar(out=OM[:, :], in0=EC[:, :], scalar1=-1.0, scalar2=1.0,
                      op0=ALU.mult, op1=ALU.add)
    vec.tensor_scalar(out=ZC[:, :], in0=EC[:, :], scalar1=float(ZSLOT),
                      scalar2=None, op0=ALU.mult)
    for r in range(6):
        sl = IDXF[:, 98 * r : 98 * r + 98]
        vec.tensor_tensor(out=sl, in0=sl, in1=OM[:, :], op=ALU.mult)
        vec.tensor_tensor(out=sl, in0=sl, in1=ZC[:, :], op=ALU.add)
    vec.tensor_copy(out=IDXU[:, :], in_=IDXF[:, :])

    # ---- table build (11 elementwise-max passes)
    def tv(t):
        return tab[:, t * HW : (t + 1) * HW].rearrange("p (h w) -> p h w", h=H)

    scr_v = SCR[:, :].rearrange("p (h w) -> p h w", h=H)
    # h-pyramid on raw: win2 scratch, then win4 -> T(0,1)
    any_.tensor_tensor(out=scr_v[:, 0:55, :], in0=tv(0)[:, 0:55, :],
                       in1=tv(0)[:, 1:56, :], op=ALU.max)
    any_.tensor_tensor(out=tv(1)[:, 0:53, :], in0=scr_v[:, 0:53, :],
                       in1=scr_v[:, 2:55, :], op=ALU.max)
    vec.memset(tv(1)[:, 53:56, :], 0.0)
    # two w-chains (khidx 0 and 1): T(kw,kh) = max(prev[w], prev[w+2^(kw-1)])
    for kh in (0, 1):
        for kw, d in ((1, 1), (2, 2), (3, 4)):
            src, dst = tv(2 * (kw - 1) + kh), tv(2 * kw + kh)
            n = W - 2 * d + 1
            any_.tensor_tensor(out=dst[:, :, 0:n], in0=src[:, :, 0:n],
                               in1=src[:, :, d : d + n], op=ALU.max)
            vec.memset(dst[:, :, n:W], 0.0)

    # ---- gathers + running max
    data_ap = tab[:, 0:TABF]
    for r in range(6):
        dst = pl[r % 3]
        nc.gpsimd.ap_gather(
            out_ap=dst[:, :],
            in_ap=data_ap,
            idxs_ap=IDXU[:, 98 * r : 98 * r + 98],
            channels=P,
            num_elems=TABF,
            d=1,
            num_idxs=NOUT,
        )
        if r == 1:
            any_.tensor_tensor(out=acc[:, :], in0=pl[0][:, :], in1=pl[1][:, :],
                               op=ALU.max)
        elif r >= 2:
            any_.tensor_tensor(out=acc[:, :], in0=acc[:, :], in1=dst[:, :],
                               op=ALU.max)

    nc.sync.dma_start(out=out_d[:, :], in_=acc[:, :])


_NC_CACHE = {}


def _get_nc():
    if "nc" not in _NC_CACHE:
        _NC_CACHE["nc"] = build_nc()
    return _NC_CACHE["nc"]


PROFILE = bool(int(os.environ.get("KERNEL_PROFILE", "0")))
LAST_EXEC_NS = [None]
LAST_RESULTS = [None]


def _ensure_profile_hook():
    """Provide the antenv.axon_hooks registry this container lacks and
    register the NTFF profile hook from trn_agent_boot (profiling only)."""
    import types

    if "antenv.axon_hooks" in sys.modules:
        return
    import antenv

    m = types.ModuleType("antenv.axon_hooks")
    m._hook = None
    m.set_axon_ntff_profile_hook = lambda h: setattr(m, "_hook", h)
    m.get_axon_ntff_profile_hook = lambda: m._hook
    sys.modules["antenv.axon_hooks"] = m
    antenv.axon_hooks = m
    try:
        from trn_agent_boot.trn_boot import _ntff_profile_via_ctypes

        h = _ntff_profile_via_ctypes("/opt/axon/libaxon_pjrt.so")
        if h is not None:
            m._hook = h
    except Exception:
        pass
    import concourse.bass_utils as bu

    bu.upload_artifacts = lambda tmpdir: "local://" + tmpdir


def kernel(features, rois):
    features = np.asarray(features, dtype=np.float32)
    rois = np.asarray(rois, dtype=np.float32)
    B, N, C = 2, 128, 128
    nc = _get_nc()
    in_maps = []
    for core in range(8):
        b, q = divmod(core, 4)
        in_maps.append({
            "features": np.ascontiguousarray(features[b]),
            "rois": np.ascontiguousarray(rois[b, 32 * q : 32 * q + 32, :]),
        })
    if PROFILE:
        _ensure_profile_hook()
    from concourse.bass_utils import run_bass_kernel_spmd

    res = run_bass_kernel_spmd(
        nc, in_maps, core_ids=list(range(8)), trace=PROFILE
    )
    LAST_EXEC_NS[0] = res.exec_time_ns
    LAST_RESULTS[0] = res
    out = np.zeros((B, N, C, 7, 7), np.float32)
    for core in range(8):
        b, q = divmod(core, 4)
        r = np.asarray(res.results[core]["out"])  # (C, 1568)
        r = r.reshape(C, 2, 7, 7, 16).transpose(1, 4, 0, 2, 3)
        out[b, 32 * q : 32 * q + 32] = r.reshape(32, C, 7, 7)
    return out
